# revision 29
# baseline (speedup 1.0000x reference)
"""Self-contained Trainium2 Bass kernel for nn_CA_9363028705415 (sparse_attention).

Computes, per batch b:
    Q = relu(x[b] @ qW1 + qb1) @ qW2 + qb2          # [M, K]
    Kt = relu(x[b] @ kW1 + kb1) @ kW2 + kb2         # [M, K]
    S = Q @ Kt.T                                    # [M, M]
    out[b] = softmax(S / rowmax(S), axis=-1)        # max-DIVISION normalization

Shapes: B=16, M=2048, D=128, H=256, K=64.  Output [16, 2048, 2048] f32 (256 MB).
Sharding: data-parallel over batch across 8 NeuronCores; 2 batches/core.

Design (vs the 180 us f32 baseline):
  - Output is fp16 on the wire (halves the HBM write to 16.8 MB/core);
    host upcasts to f32.
  - x is pre-transposed + bf16-cast on the host -> no PE transposes, no
    DVE casts, xT DMAs straight into SBUF.  Weights/biases pre-packed.
  - Per 128-row S tile: PE matmuls -> DVE evac+row-max (PSUM f32 is a
    1x-rate read; 2.3us) -> ACT exp fp16->fp16 with fused 1/max scale
    and row-sum accum (2.1us+0.28 accum read; the only exp engine) ->
    norm multiply by 1/rowsum on DVE (4x fp16, 0.77us) or ACT per
    NORM_PATTERN (balances the two ~103us engine budgets).
  - The 1/max reciprocal is its own [128,1] DVE op so exp(rt) depends
    only on its own evac, never on exp(rt-1)'s accumulator.  Row-sums of
    tile pairs (2p, 2p+1) share a [128,2] tile (both columns written by
    ACT accum-reads, in-order) so one reciprocal serves two norms; norms
    lag 2-3 tiles behind the exp.
  - ACT exp table pre-loaded via a dummy exp at t=0; ~14 dummy matmuls
    warm the PE HAM clock gate during the initial x DMA.

Known walls (hardware-measured): DVE evac+max is a 1x PSUM read
(2.29us/tile, 73us/core) and ACT exp is 1x (2.36us/tile, 76us/core);
with norms/recips/MLP-evacs layered on, both engines budget ~103us.
The PE sticks at half clock (HAM K=4/8) for the back half of every run
regardless of occupancy, making the matmul stream a ~143us co-pacer.
Failed experiments: GPSIMD norms (29.6us/tile + starves DVE via the
shared SBUF port), >512-wide matmuls (walrus s3d3 assert), filler
matmuls to unstick the HAM (net loss), ldweights=False (ignored).
"""

import numpy as np
import ml_dtypes

import concourse.bass as bass
import concourse.mybir as mybir
from concourse import bacc
import concourse.tile as tile
from concourse.bass import ts
from concourse.bass_utils import run_bass_kernel_spmd

F32 = mybir.dt.float32
BF16 = mybir.dt.bfloat16
FP16 = mybir.dt.float16
AF = mybir.ActivationFunctionType
ALU = mybir.AluOpType

N_CORES = 8
B, M, D, H, KF = 16, 2048, 128, 256, 64
BPC = B // N_CORES     # batches per core
MT = M // 128          # 16 row-tiles per batch
FC = M // 512          # 4 matmul free-chunks of 512
PAIR = 2               # row-tiles per output DMA (1 MB bf16 chunks)

# norm engine per row-tile (32 tiles/core).  DVE 16-bit tensor_scalar runs
# ~0.77us; ACT is 2.08us but has slack -- a few "act" entries rebalance.
# GPSIMD measured 29.6us/tile AND starves DVE via the shared SBUF port: never.
NORM_PATTERN = (
    "dve", "dve", "dve", "dve", "dve", "dve", "dve", "dve",
    "act", "dve", "dve", "dve", "dve", "act", "dve", "dve",
) * 2
# engines for the 4 MLP1 evacs per batch (PSUM f32 -> bf16, relu+bias)
MLP1_EVAC = ("act", "act", "act", "act")
# engine for the MLP2 evac per head
MLP2_EVAC = {"q": "act", "k": "act"}


def _evac_bias(nc, engine, out, in_, bias, relu):
    """out = [relu](in_ + bias), bias is [P,1] per-partition AP."""
    if engine == "act":
        nc.scalar.activation(
            out, in_, AF.Relu if relu else AF.Identity, bias=bias, scale=1.0
        )
    else:
        if relu:
            nc.vector.tensor_scalar(out, in_, bias, 0.0, op0=ALU.add, op1=ALU.max)
        else:
            nc.vector.tensor_scalar(out, in_, bias, None, op0=ALU.add)


def _norm(nc, engine, out, t, isum):
    if engine == "act":
        nc.scalar.mul(out, t, isum)
    elif engine == "gps":
        nc.gpsimd.tensor_scalar_mul(out, t, isum)
    else:
        nc.vector.tensor_scalar_mul(out, t, isum)


def build_nc():
    nc = bacc.Bacc()

    # host-prepped inputs: xT per batch [D, M] bf16; weights bf16; biases f32
    xt = nc.dram_tensor("xt", [BPC, D, M], BF16, kind="ExternalInput")
    w1d, b1d, w2d, b2d = {}, {}, {}, {}
    for h in ("q", "k"):
        w1d[h] = nc.dram_tensor(f"{h}W1", [D, H], BF16, kind="ExternalInput")
        b1d[h] = nc.dram_tensor(f"{h}b1", [128, 2], F32, kind="ExternalInput")
        w2d[h] = nc.dram_tensor(f"{h}W2", [128, 2, KF], BF16, kind="ExternalInput")
        b2d[h] = nc.dram_tensor(f"{h}b2", [KF, 1], F32, kind="ExternalInput")
    out = nc.dram_tensor("out", [BPC, M, M], FP16, kind="ExternalOutput")

    # [b, p, n, m]: out[b, n*128+p, m]
    out_r = out[:].rearrange("b (n p) m -> b p n m", p=128)

    with tile.TileContext(nc) as tc:
        with (
            tc.tile_pool(name="consts", bufs=1) as consts,
            tc.tile_pool(name="xt", bufs=2) as xt_pool,
            tc.tile_pool(name="ht", bufs=2) as ht_pool,
            tc.tile_pool(name="qkt", bufs=2) as qkt_pool,
            tc.tile_pool(name="sc", bufs=4) as sc_pool,
            tc.tile_pool(name="texp", bufs=5) as t_pool,
            tc.tile_pool(name="osb", bufs=3) as out_pool,
            tc.tile_pool(name="small", bufs=6) as small_pool,
            tc.tile_pool(name="psum", bufs=2, space="PSUM") as psum_pool,
        ):
            norm_i = 0

            # ---- ACT exp-table preload: dummy exp before anything else ----
            dummy = consts.tile([128, 1], F32, tag="dummy")
            dummy2 = consts.tile([128, 1], F32, tag="dummy2")
            nc.vector.memset(dummy, 0.0)
            nc.scalar.activation(dummy2, dummy, AF.Exp, bias=0.0, scale=1.0)

            # ---- PE HAM warmup: ~14 small dummy matmuls during the x DMA ----
            wsb = consts.tile([128, 64], BF16, tag="wsb")
            nc.vector.memset(wsb, 0.5)
            wps = psum_pool.tile([64, 64], F32, tag="ps", name="wps")
            for _ in range(14):
                nc.tensor.matmul(wps, lhsT=wsb, rhs=wsb, start=True, stop=True)


            # ---- input DMAs: batch-0 xT first, consts, then batch-1 xT ----
            xf = {}
            for b in range(BPC):
                xf[b] = xt_pool.tile([128, M], BF16, tag=f"xf{b}", name="xf")
            nc.sync.dma_start(out=xf[0], in_=xt[0])

            w1, w2, b1, b2 = {}, {}, {}, {}
            for h in ("q", "k"):
                w1[h] = consts.tile([D, H], BF16, tag=f"w1{h}", name=f"w1{h}")
                nc.sync.dma_start(out=w1[h], in_=w1d[h][:])
                w2[h] = consts.tile([128, 2, KF], BF16, tag=f"w2{h}", name=f"w2{h}")
                nc.sync.dma_start(out=w2[h], in_=w2d[h][:])
                b1[h] = consts.tile([128, 2], F32, tag=f"b1{h}", name=f"b1{h}")
                nc.sync.dma_start(out=b1[h], in_=b1d[h][:])
                b2[h] = consts.tile([KF, 1], F32, tag=f"b2{h}", name=f"b2{h}")
                nc.sync.dma_start(out=b2[h], in_=b2d[h][:])
            nc.sync.dma_start(out=xf[1], in_=xt[1])

            def phase_a_chunks(b, fast=False):
                """Emit-chunks for batch b's MLP pipeline (no transposes:
                xT comes pre-transposed from the host)."""
                ctx = {}
                mlp1_i = [0]

                def c_mlp1(h, pc):
                    def go():
                        if ("ht", h) not in ctx:
                            ctx[("ht", h)] = ht_pool.tile(
                                [128, 2, M], BF16, tag=f"ht{h}", name=f"ht{h}"
                            )
                        ps1 = psum_pool.tile([128, M], F32, tag="ps", name="ps1")
                        for fc in range(FC):
                            nc.tensor.matmul(
                                ps1[:, ts(fc, 512)],
                                lhsT=w1[h][:, ts(pc, 128)],
                                rhs=xf[b][:, ts(fc, 512)],
                                start=True,
                                stop=True,
                            )
                        if fast:
                            for fc in range(FC):
                                _evac_bias(
                                    nc,
                                    ("act", "dve")[fc % 2],
                                    ctx[("ht", h)][:, pc, ts(fc, 512)],
                                    ps1[:, ts(fc, 512)],
                                    b1[h][:, pc : pc + 1],
                                    relu=True,
                                )
                        else:
                            e = MLP1_EVAC[mlp1_i[0] % len(MLP1_EVAC)]
                            mlp1_i[0] += 1
                            _evac_bias(
                                nc,
                                e,
                                ctx[("ht", h)][:, pc, :],
                                ps1,
                                b1[h][:, pc : pc + 1],
                                relu=True,
                            )
                    return go

                def c_mlp2(h):
                    def go():
                        ps2 = psum_pool.tile([KF, M], F32, tag="ps", name="ps2")
                        for fc in range(FC):
                            for kc in range(2):
                                nc.tensor.matmul(
                                    ps2[:, ts(fc, 512)],
                                    lhsT=w2[h][:, kc, :],
                                    rhs=ctx[("ht", h)][:, kc, ts(fc, 512)],
                                    start=(kc == 0),
                                    stop=(kc == 1),
                                )
                        q = qkt_pool.tile([KF, M], BF16, tag=f"qkt{h}", name=f"qkt{h}")
                        ctx[("qkt", h)] = q
                        if fast:
                            for fc in range(FC):
                                _evac_bias(
                                    nc,
                                    ("act", "dve")[fc % 2],
                                    q[:, ts(fc, 512)],
                                    ps2[:, ts(fc, 512)],
                                    b2[h],
                                    relu=False,
                                )
                        else:
                            _evac_bias(
                                nc, MLP2_EVAC[h], q, ps2, b2[h], relu=False
                            )
                    return go

                chunks = []
                for pc in range(2):
                    chunks.append(c_mlp1("q", pc))
                    chunks.append(c_mlp1("k", pc))
                chunks.append(c_mlp2("q"))
                chunks.append(c_mlp2("k"))
                return ctx, chunks

            def s_loop(b, qkt, next_chunks):
                """S + softmax loop for batch b, interleaving next batch's
                MLP chunks into the early iterations."""
                nonlocal norm_i
                osb_tiles = {}
                pending = None

                def finish(j, t_j, isum_ap):
                    nonlocal norm_i
                    _norm(
                        nc,
                        NORM_PATTERN[norm_i % len(NORM_PATTERN)],
                        osb_tiles[j // PAIR][:, ts(j % PAIR, M)],
                        t_j,
                        isum_ap,
                    )
                    norm_i += 1
                    if j % PAIR == PAIR - 1:
                        osb = osb_tiles.pop(j // PAIR)
                        if j == MT - 1:
                            for jj in range(PAIR):
                                nc.sync.dma_start(
                                    out=out_r[b][:, j - PAIR + 1 + jj : j - PAIR + 2 + jj, :],
                                    in_=osb[:, ts(jj, M)],
                                )
                        else:
                            nc.sync.dma_start(
                                out=out_r[b][:, j - PAIR + 1 : j + 1, :],
                                in_=osb,
                            )

                # maxes[rt] = row-max(rt); sums[rt] = exp-row-sum(rt).  The
                # max reciprocal is a separate instruction from the sum
                # reciprocal so exp(rt) never waits on exp(rt-1)'s
                # accumulator (keeps the recip off the ACT critical chain).
                # row-sums of tiles (2p, 2p+1) share one [128,2] tile so a
                # single reciprocal serves both norms.  Both columns are
                # written by ACT accum-reads (in-order on ACT), so the DVE
                # reciprocal's wait on the later column covers the earlier.
                maxes, spair, t_hist = {}, {}, {}
                for rt in range(MT):
                    ps_s = psum_pool.tile([128, M], F32, tag="ps", name="ps_s")
                    for fc in range(FC):
                        nc.tensor.matmul(
                            ps_s[:, ts(fc, 512)],
                            lhsT=qkt["q"][:, ts(rt, 128)],
                            rhs=qkt["k"][:, ts(fc, 512)],
                            start=True,
                            stop=True,
                        )
                    # PSUM f32 -> SBUF fp16 with fused row-max; frees the
                    # PSUM slot so exp reads the SBUF copy.
                    sc_t = sc_pool.tile([128, M], FP16, tag="sc", name="sc")
                    maxes[rt] = small_pool.tile([128, 1], F32, tag="mx", name="mx")
                    nc.vector.tensor_scalar(
                        sc_t,
                        ps_s,
                        0.0,
                        None,
                        op0=ALU.add,
                        op1=ALU.max,
                        accum_out=maxes[rt],
                    )

                    imax = small_pool.tile([128, 1], F32, tag="im", name="imax")
                    nc.vector.reciprocal(imax, maxes[rt])

                    t_t = t_pool.tile([128, M], FP16, tag="t")
                    t_hist[rt] = t_t
                    if rt % 2 == 0:
                        spair[rt // 2] = small_pool.tile(
                            [128, 2], F32, tag="sm", name="sm"
                        )
                    nc.scalar.activation(
                        t_t,
                        sc_t,
                        AF.Exp,
                        bias=0.0,
                        scale=imax,
                        accum_out=spair[rt // 2][:, rt % 2 : rt % 2 + 1],
                    )

                    if rt % PAIR == 0:
                        osb_tiles[rt // PAIR] = out_pool.tile(
                            [128, PAIR * M], FP16, tag="o", name="osb"
                        )
                    if rt >= 3 and rt % 2 == 1:
                        p = (rt - 3) // 2
                        isp = small_pool.tile([128, 2], F32, tag="is", name="isum")
                        nc.vector.reciprocal(isp, spair[p])
                        finish(2 * p, t_hist.pop(2 * p), isp[:, 0:1])
                        finish(2 * p + 1, t_hist.pop(2 * p + 1), isp[:, 1:2])

                    if next_chunks:
                        next_chunks.pop(0)()
                p = MT // 2 - 1
                isp = small_pool.tile([128, 2], F32, tag="is", name="isum")
                nc.vector.reciprocal(isp, spair[p])
                finish(2 * p, t_hist.pop(2 * p), isp[:, 0:1])
                finish(2 * p + 1, t_hist.pop(2 * p + 1), isp[:, 1:2])
                while next_chunks:
                    next_chunks.pop(0)()

            ctx0, chunks0 = phase_a_chunks(0, fast=True)
            for c in chunks0:
                c()
            qkt0 = {"q": ctx0[("qkt", "q")], "k": ctx0[("qkt", "k")]}

            ctx1, chunks1 = phase_a_chunks(1)
            s_loop(0, qkt0, chunks1)
            qkt1 = {"q": ctx1[("qkt", "q")], "k": ctx1[("qkt", "k")]}
            s_loop(1, qkt1, [])
    nc.finalize()
    return nc


_NC_CACHE = None


def _get_nc():
    global _NC_CACHE
    if _NC_CACHE is None:
        _NC_CACHE = build_nc()
    return _NC_CACHE


def _prep_weights(inputs):
    """Host-side packing (shared across cores)."""
    bf = ml_dtypes.bfloat16
    wm = {}
    for h in ("q", "k"):
        wm[f"{h}W1"] = np.ascontiguousarray(inputs[f"{h}W1"], dtype=bf)
        # [H] -> [128, 2] with h-index = c*128+p
        wm[f"{h}b1"] = np.ascontiguousarray(
            np.asarray(inputs[f"{h}b1"], dtype=np.float32).reshape(2, 128).T
        )
        # [H, K] -> [128, 2, K] with h-index = c*128+p
        wm[f"{h}W2"] = np.ascontiguousarray(
            np.asarray(inputs[f"{h}W2"], dtype=bf).reshape(2, 128, KF).transpose(1, 0, 2)
        )
        wm[f"{h}b2"] = np.ascontiguousarray(
            np.asarray(inputs[f"{h}b2"], dtype=np.float32).reshape(KF, 1)
        )
    return wm


def run(inputs, trace=False, trace_cores=None):
    """Run on 8 cores; returns (full_output [B,M,M] f32, BassKernelResults)."""
    nc = _get_nc()
    bf = ml_dtypes.bfloat16
    x = np.asarray(inputs["x"], dtype=np.float32)
    # [B, M, D] -> [B, D, M] bf16, per-core slices
    xT = np.ascontiguousarray(x.transpose(0, 2, 1).astype(bf))
    wm = _prep_weights(inputs)
    in_maps = []
    for c in range(N_CORES):
        im = {"xt": np.ascontiguousarray(xT[c * BPC : (c + 1) * BPC])}
        im.update(wm)
        in_maps.append(im)
    res = run_bass_kernel_spmd(
        nc,
        in_maps,
        core_ids=list(range(N_CORES)),
        trace=trace,
        trace_cores=trace_cores,
    )
    full = np.empty((B, M, M), dtype=np.float32)
    for c in range(N_CORES):
        full[c * BPC : (c + 1) * BPC] = res.results[c]["out"].astype(np.float32)
    assert full.shape == (B, M, M) and full.dtype == np.float32
    return full, res


def kernel(**inputs) -> np.ndarray:
    out, _ = run(inputs, trace=False)
    return out


# revision 30
# speedup vs baseline: 1.0094x; 1.0094x over previous
"""Self-contained Trainium2 Bass kernel for nn_CA_9363028705415 (sparse_attention).

Computes, per batch b:
    Q = relu(x[b] @ qW1 + qb1) @ qW2 + qb2          # [M, K]
    Kt = relu(x[b] @ kW1 + kb1) @ kW2 + kb2         # [M, K]
    S = Q @ Kt.T                                    # [M, M]
    out[b] = softmax(S / rowmax(S), axis=-1)        # max-DIVISION normalization

Shapes: B=16, M=2048, D=128, H=256, K=64.  Output [16, 2048, 2048] f32 (256 MB).
Sharding: data-parallel over batch across 8 NeuronCores; 2 batches/core.

Design (vs the 180 us f32 baseline):
  - Output is fp16 on the wire (halves the HBM write to 16.8 MB/core);
    host upcasts to f32.
  - x is pre-transposed + bf16-cast on the host -> no PE transposes, no
    DVE casts, xT DMAs straight into SBUF.  Weights/biases pre-packed.
  - Per 128-row S tile: PE matmuls -> DVE evac+row-max (PSUM f32 is a
    1x-rate read; 2.3us) -> ACT exp fp16->fp16 with fused 1/max scale
    and row-sum accum (2.1us+0.28 accum read; the only exp engine) ->
    norm multiply by 1/rowsum on DVE (4x fp16, 0.77us) or ACT per
    NORM_PATTERN (balances the two ~103us engine budgets).
  - The 1/max reciprocal is its own [128,1] DVE op so exp(rt) depends
    only on its own evac, never on exp(rt-1)'s accumulator.  Row-sums of
    tile pairs (2p, 2p+1) share a [128,2] tile (both columns written by
    ACT accum-reads, in-order) so one reciprocal serves two norms; norms
    lag 2-3 tiles behind the exp.
  - ACT exp table pre-loaded via a dummy exp at t=0; ~14 dummy matmuls
    warm the PE HAM clock gate during the initial x DMA.

Known walls (hardware-measured): DVE evac+max is a 1x PSUM read
(2.29us/tile, 73us/core) and ACT exp is 1x (2.36us/tile, 76us/core);
with norms/recips/MLP-evacs layered on, both engines budget ~103us.
The PE sticks at half clock (HAM K=4/8) for the back half of every run
regardless of occupancy, making the matmul stream a ~143us co-pacer.
Failed experiments: GPSIMD norms (29.6us/tile + starves DVE via the
shared SBUF port), >512-wide matmuls (walrus s3d3 assert), filler
matmuls to unstick the HAM (net loss), ldweights=False (ignored).
"""

import numpy as np
import ml_dtypes

import concourse.bass as bass
import concourse.mybir as mybir
from concourse import bacc
import concourse.tile as tile
from concourse.bass import ts
from concourse.bass_utils import run_bass_kernel_spmd

F32 = mybir.dt.float32
BF16 = mybir.dt.bfloat16
FP16 = mybir.dt.float16
AF = mybir.ActivationFunctionType
ALU = mybir.AluOpType

N_CORES = 8
B, M, D, H, KF = 16, 2048, 128, 256, 64
BPC = B // N_CORES     # batches per core
MT = M // 128          # 16 row-tiles per batch
FC = M // 512          # 4 matmul free-chunks of 512
PAIR = 2               # row-tiles per output DMA (1 MB bf16 chunks)

# norm engine per row-tile (32 tiles/core).  DVE 16-bit tensor_scalar runs
# ~0.77us; ACT is 2.08us but has slack -- a few "act" entries rebalance.
# GPSIMD measured 29.6us/tile AND starves DVE via the shared SBUF port: never.
NORM_PATTERN = (
    "dve", "dve", "dve", "act", "dve", "dve", "dve", "dve",
    "dve", "dve", "dve", "act", "dve", "dve", "dve", "dve",
) * 2
# engines for the 4 MLP1 evacs per batch (PSUM f32 -> bf16, relu+bias)
MLP1_EVAC = ("act", "act", "act", "act")
# engine for the MLP2 evac per head
MLP2_EVAC = {"q": "act", "k": "act"}


def _evac_bias(nc, engine, out, in_, bias, relu):
    """out = [relu](in_ + bias), bias is [P,1] per-partition AP."""
    if engine == "act":
        nc.scalar.activation(
            out, in_, AF.Relu if relu else AF.Identity, bias=bias, scale=1.0
        )
    else:
        if relu:
            nc.vector.tensor_scalar(out, in_, bias, 0.0, op0=ALU.add, op1=ALU.max)
        else:
            nc.vector.tensor_scalar(out, in_, bias, None, op0=ALU.add)


def _norm(nc, engine, out, t, isum):
    if engine == "act":
        nc.scalar.mul(out, t, isum)
    elif engine == "gps":
        nc.gpsimd.tensor_scalar_mul(out, t, isum)
    else:
        nc.vector.tensor_scalar_mul(out, t, isum)


def build_nc():
    nc = bacc.Bacc()

    # host-prepped inputs: xT per batch [D, M] bf16; weights bf16; biases f32
    xt = nc.dram_tensor("xt", [BPC, D, M], BF16, kind="ExternalInput")
    w1d, b1d, w2d, b2d = {}, {}, {}, {}
    for h in ("q", "k"):
        w1d[h] = nc.dram_tensor(f"{h}W1", [D, H], BF16, kind="ExternalInput")
        b1d[h] = nc.dram_tensor(f"{h}b1", [128, 2], F32, kind="ExternalInput")
        w2d[h] = nc.dram_tensor(f"{h}W2", [128, 2, KF], BF16, kind="ExternalInput")
        b2d[h] = nc.dram_tensor(f"{h}b2", [KF, 1], F32, kind="ExternalInput")
    out = nc.dram_tensor("out", [BPC, M, M], FP16, kind="ExternalOutput")

    # [b, p, n, m]: out[b, n*128+p, m]
    out_r = out[:].rearrange("b (n p) m -> b p n m", p=128)

    with tile.TileContext(nc) as tc:
        with (
            tc.tile_pool(name="consts", bufs=1) as consts,
            tc.tile_pool(name="xt", bufs=2) as xt_pool,
            tc.tile_pool(name="ht", bufs=2) as ht_pool,
            tc.tile_pool(name="qkt", bufs=2) as qkt_pool,
            tc.tile_pool(name="sc", bufs=3) as sc_pool,
            tc.tile_pool(name="texp", bufs=5) as t_pool,
            tc.tile_pool(name="osb", bufs=3) as out_pool,
            tc.tile_pool(name="small", bufs=6) as small_pool,
            tc.tile_pool(name="psum", bufs=2, space="PSUM") as psum_pool,
        ):
            norm_i = 0

            # ---- ACT exp-table preload: dummy exp before anything else ----
            dummy = consts.tile([128, 1], F32, tag="dummy")
            dummy2 = consts.tile([128, 1], F32, tag="dummy2")
            nc.vector.memset(dummy, 0.0)
            nc.scalar.activation(dummy2, dummy, AF.Exp, bias=0.0, scale=1.0)

            # ---- PE HAM warmup: ~14 small dummy matmuls during the x DMA ----
            wsb = consts.tile([128, 64], BF16, tag="wsb")
            nc.vector.memset(wsb, 0.5)
            wps = psum_pool.tile([64, 64], F32, tag="ps", name="wps")
            for _ in range(14):
                nc.tensor.matmul(wps, lhsT=wsb, rhs=wsb, start=True, stop=True)


            # ---- input DMAs: batch-0 xT first, consts, then batch-1 xT ----
            xf = {}
            for b in range(BPC):
                xf[b] = xt_pool.tile([128, M], BF16, tag=f"xf{b}", name="xf")
            nc.sync.dma_start(out=xf[0], in_=xt[0])

            w1, w2, b1, b2 = {}, {}, {}, {}
            for h in ("q", "k"):
                w1[h] = consts.tile([D, H], BF16, tag=f"w1{h}", name=f"w1{h}")
                nc.sync.dma_start(out=w1[h], in_=w1d[h][:])
                w2[h] = consts.tile([128, 2, KF], BF16, tag=f"w2{h}", name=f"w2{h}")
                nc.sync.dma_start(out=w2[h], in_=w2d[h][:])
                b1[h] = consts.tile([128, 2], F32, tag=f"b1{h}", name=f"b1{h}")
                nc.sync.dma_start(out=b1[h], in_=b1d[h][:])
                b2[h] = consts.tile([KF, 1], F32, tag=f"b2{h}", name=f"b2{h}")
                nc.sync.dma_start(out=b2[h], in_=b2d[h][:])
            nc.sync.dma_start(out=xf[1], in_=xt[1])

            def phase_a_chunks(b, fast=False):
                """Emit-chunks for batch b's MLP pipeline (no transposes:
                xT comes pre-transposed from the host)."""
                ctx = {}
                mlp1_i = [0]

                def c_mlp1(h, pc):
                    def go():
                        if ("ht", h) not in ctx:
                            ctx[("ht", h)] = ht_pool.tile(
                                [128, 2, M], BF16, tag=f"ht{h}", name=f"ht{h}"
                            )
                        ps1 = psum_pool.tile([128, M], F32, tag="ps", name="ps1")
                        for fc in range(FC):
                            nc.tensor.matmul(
                                ps1[:, ts(fc, 512)],
                                lhsT=w1[h][:, ts(pc, 128)],
                                rhs=xf[b][:, ts(fc, 512)],
                                start=True,
                                stop=True,
                            )
                        if fast:
                            for fc in range(FC):
                                _evac_bias(
                                    nc,
                                    ("act", "dve")[fc % 2],
                                    ctx[("ht", h)][:, pc, ts(fc, 512)],
                                    ps1[:, ts(fc, 512)],
                                    b1[h][:, pc : pc + 1],
                                    relu=True,
                                )
                        else:
                            e = MLP1_EVAC[mlp1_i[0] % len(MLP1_EVAC)]
                            mlp1_i[0] += 1
                            _evac_bias(
                                nc,
                                e,
                                ctx[("ht", h)][:, pc, :],
                                ps1,
                                b1[h][:, pc : pc + 1],
                                relu=True,
                            )
                    return go

                def c_mlp2(h):
                    def go():
                        ps2 = psum_pool.tile([KF, M], F32, tag="ps", name="ps2")
                        for fc in range(FC):
                            for kc in range(2):
                                nc.tensor.matmul(
                                    ps2[:, ts(fc, 512)],
                                    lhsT=w2[h][:, kc, :],
                                    rhs=ctx[("ht", h)][:, kc, ts(fc, 512)],
                                    start=(kc == 0),
                                    stop=(kc == 1),
                                )
                        q = qkt_pool.tile([KF, M], BF16, tag=f"qkt{h}", name=f"qkt{h}")
                        ctx[("qkt", h)] = q
                        if fast:
                            for fc in range(FC):
                                _evac_bias(
                                    nc,
                                    ("act", "dve")[fc % 2],
                                    q[:, ts(fc, 512)],
                                    ps2[:, ts(fc, 512)],
                                    b2[h],
                                    relu=False,
                                )
                        else:
                            _evac_bias(
                                nc, MLP2_EVAC[h], q, ps2, b2[h], relu=False
                            )
                    return go

                chunks = []
                for pc in range(2):
                    chunks.append(c_mlp1("q", pc))
                    chunks.append(c_mlp1("k", pc))
                chunks.append(c_mlp2("q"))
                chunks.append(c_mlp2("k"))
                return ctx, chunks

            def s_loop(b, qkt, next_chunks):
                """S + softmax loop for batch b, interleaving next batch's
                MLP chunks into the early iterations."""
                nonlocal norm_i
                osb_tiles = {}
                pending = None

                def finish(j, t_j, isum_ap):
                    nonlocal norm_i
                    _norm(
                        nc,
                        NORM_PATTERN[norm_i % len(NORM_PATTERN)],
                        osb_tiles[j // PAIR][:, ts(j % PAIR, M)],
                        t_j,
                        isum_ap,
                    )
                    norm_i += 1
                    if j % PAIR == PAIR - 1:
                        osb = osb_tiles.pop(j // PAIR)
                        if j == MT - 1:
                            for jj in range(PAIR):
                                nc.sync.dma_start(
                                    out=out_r[b][:, j - PAIR + 1 + jj : j - PAIR + 2 + jj, :],
                                    in_=osb[:, ts(jj, M)],
                                )
                        else:
                            nc.sync.dma_start(
                                out=out_r[b][:, j - PAIR + 1 : j + 1, :],
                                in_=osb,
                            )

                # maxes[rt] = row-max(rt); sums[rt] = exp-row-sum(rt).  The
                # max reciprocal is a separate instruction from the sum
                # reciprocal so exp(rt) never waits on exp(rt-1)'s
                # accumulator (keeps the recip off the ACT critical chain).
                # row-sums of tiles (2p, 2p+1) share one [128,2] tile so a
                # single reciprocal serves both norms.  Both columns are
                # written by ACT accum-reads (in-order on ACT), so the DVE
                # reciprocal's wait on the later column covers the earlier.
                maxes, spair, t_hist = {}, {}, {}
                for rt in range(MT):
                    ps_s = psum_pool.tile([128, M], F32, tag="ps", name="ps_s")
                    for fc in range(FC):
                        nc.tensor.matmul(
                            ps_s[:, ts(fc, 512)],
                            lhsT=qkt["q"][:, ts(rt, 128)],
                            rhs=qkt["k"][:, ts(fc, 512)],
                            start=True,
                            stop=True,
                        )
                    # PSUM f32 -> SBUF fp16 with fused row-max; frees the
                    # PSUM slot so exp reads the SBUF copy.
                    sc_t = sc_pool.tile([128, M], FP16, tag="sc", name="sc")
                    maxes[rt] = small_pool.tile([128, 1], F32, tag="mx", name="mx")
                    nc.vector.tensor_scalar(
                        sc_t,
                        ps_s,
                        0.0,
                        None,
                        op0=ALU.add,
                        op1=ALU.max,
                        accum_out=maxes[rt],
                    )

                    imax = small_pool.tile([128, 1], F32, tag="im", name="imax")
                    nc.vector.reciprocal(imax, maxes[rt])

                    t_t = t_pool.tile([128, M], FP16, tag="t")
                    t_hist[rt] = t_t
                    if rt % 2 == 0:
                        spair[rt // 2] = small_pool.tile(
                            [128, 2], F32, tag="sm", name="sm"
                        )
                    nc.scalar.activation(
                        t_t,
                        sc_t,
                        AF.Exp,
                        bias=0.0,
                        scale=imax,
                        accum_out=spair[rt // 2][:, rt % 2 : rt % 2 + 1],
                    )

                    if rt % PAIR == 0:
                        osb_tiles[rt // PAIR] = out_pool.tile(
                            [128, PAIR * M], FP16, tag="o", name="osb"
                        )
                    if rt >= 3 and rt % 2 == 1:
                        p = (rt - 3) // 2
                        isp = small_pool.tile([128, 2], F32, tag="is", name="isum")
                        nc.vector.reciprocal(isp, spair[p])
                        finish(2 * p, t_hist.pop(2 * p), isp[:, 0:1])
                        finish(2 * p + 1, t_hist.pop(2 * p + 1), isp[:, 1:2])

                    if next_chunks:
                        next_chunks.pop(0)()
                p = MT // 2 - 1
                isp = small_pool.tile([128, 2], F32, tag="is", name="isum")
                nc.vector.reciprocal(isp, spair[p])
                finish(2 * p, t_hist.pop(2 * p), isp[:, 0:1])
                finish(2 * p + 1, t_hist.pop(2 * p + 1), isp[:, 1:2])
                while next_chunks:
                    next_chunks.pop(0)()

            ctx0, chunks0 = phase_a_chunks(0, fast=True)
            for c in chunks0:
                c()
            qkt0 = {"q": ctx0[("qkt", "q")], "k": ctx0[("qkt", "k")]}

            ctx1, chunks1 = phase_a_chunks(1)
            s_loop(0, qkt0, chunks1)
            qkt1 = {"q": ctx1[("qkt", "q")], "k": ctx1[("qkt", "k")]}
            s_loop(1, qkt1, [])
    nc.finalize()
    return nc


_NC_CACHE = None


def _get_nc():
    global _NC_CACHE
    if _NC_CACHE is None:
        _NC_CACHE = build_nc()
    return _NC_CACHE


def _prep_weights(inputs):
    """Host-side packing (shared across cores)."""
    bf = ml_dtypes.bfloat16
    wm = {}
    for h in ("q", "k"):
        wm[f"{h}W1"] = np.ascontiguousarray(inputs[f"{h}W1"], dtype=bf)
        # [H] -> [128, 2] with h-index = c*128+p
        wm[f"{h}b1"] = np.ascontiguousarray(
            np.asarray(inputs[f"{h}b1"], dtype=np.float32).reshape(2, 128).T
        )
        # [H, K] -> [128, 2, K] with h-index = c*128+p
        wm[f"{h}W2"] = np.ascontiguousarray(
            np.asarray(inputs[f"{h}W2"], dtype=bf).reshape(2, 128, KF).transpose(1, 0, 2)
        )
        wm[f"{h}b2"] = np.ascontiguousarray(
            np.asarray(inputs[f"{h}b2"], dtype=np.float32).reshape(KF, 1)
        )
    return wm


def run(inputs, trace=False, trace_cores=None):
    """Run on 8 cores; returns (full_output [B,M,M] f32, BassKernelResults)."""
    nc = _get_nc()
    bf = ml_dtypes.bfloat16
    x = np.asarray(inputs["x"], dtype=np.float32)
    # [B, M, D] -> [B, D, M] bf16, per-core slices
    xT = np.ascontiguousarray(x.transpose(0, 2, 1).astype(bf))
    wm = _prep_weights(inputs)
    in_maps = []
    for c in range(N_CORES):
        im = {"xt": np.ascontiguousarray(xT[c * BPC : (c + 1) * BPC])}
        im.update(wm)
        in_maps.append(im)
    res = run_bass_kernel_spmd(
        nc,
        in_maps,
        core_ids=list(range(N_CORES)),
        trace=trace,
        trace_cores=trace_cores,
    )
    full = np.empty((B, M, M), dtype=np.float32)
    for c in range(N_CORES):
        full[c * BPC : (c + 1) * BPC] = res.results[c]["out"].astype(np.float32)
    assert full.shape == (B, M, M) and full.dtype == np.float32
    return full, res


def kernel(**inputs) -> np.ndarray:
    out, _ = run(inputs, trace=False)
    return out
